# revision 6
# baseline (speedup 1.0000x reference)
"""LocationAttention on 8 Trainium2 NeuronCores — pure data parallel over batch.

Math (per batch b):
  pq   = query @ W_query                                  (1, 128)
  loc2 = conv1d(att_cat, conv_kernel, SAME) @ W_loc       (T, 128)
  e    = tanh(pq + loc2 + pm) @ v                         (T,)
  w    = softmax(e)                                       (T,)
  ctx  = w @ memory                                       (512,)

Device layout is A-major (ATT_DIM=128 on partitions):
  - host folds W_loc into the conv kernel (K2 = einsum('kcf,fa->kca')) and
    im2cols the tiny (T,2) conv input into xs[kc,t] so conv+projection is ONE
    matmul per T-tile: psum[a,t] += K2[kc,a].T @ xs[kc,t]
  - pm arrives T-major; PE transpose accumulates pm.T into the same PSUM tile
  - ACT computes tanh(psum + pq) with pq as a fused per-partition bias,
    writing tanh_T (a, t)
  - energies: one PE matvec v.T @ tanh_T -> (1, T)
  - softmax without max-subtraction (|e| <= sum|v| is small); ACT Exp with
    fused accum_out row-sum; DVE reciprocal + tensor_scalar_mul
  - context: PE matvec w_col.T @ mem_tile accumulated over T tiles
"""

import numpy as np

import concourse.bass as bass
import concourse.tile as tile
from concourse import bacc, mybir
from concourse.bass_utils import run_bass_kernel_spmd

B, T, ENC, QDIM = 128, 1024, 512, 1024
A, F, KW = 128, 32, 31
NCORES = 8
BS = B // NCORES          # 16 batches per core
PAD = KW // 2             # 15
KC = KW * 2               # 62 (tap, channel) pairs
KCH = QDIM // 128         # 8 K-chunks for the query projection
NT = T // 128             # 8 T-tiles per batch
F32 = mybir.dt.float32
AF = mybir.ActivationFunctionType


def build_nc():
    # Bacc (not plain Bass): its compile() pipeline legalizes multi-sem waits
    # (move_matmul_waits_to_ldweights, event semaphores) which walrus requires.
    nc = bacc.Bacc(None, target_bir_lowering=False)
    xs_d = nc.dram_tensor("xs", [BS, KC, T], F32, kind="ExternalInput")
    pm_d = nc.dram_tensor("pm", [BS, T, A], F32, kind="ExternalInput")
    mem_d = nc.dram_tensor("mem", [BS, T, ENC], F32, kind="ExternalInput")
    qt_d = nc.dram_tensor("qt", [QDIM, BS], F32, kind="ExternalInput")
    wq_d = nc.dram_tensor("wq", [QDIM, A], F32, kind="ExternalInput")
    k2_d = nc.dram_tensor("k2", [KC, A], F32, kind="ExternalInput")
    v_d = nc.dram_tensor("v", [A, 1], F32, kind="ExternalInput")
    id_d = nc.dram_tensor("ident", [128, 128], F32, kind="ExternalInput")
    ctx_d = nc.dram_tensor("ctx", [BS, ENC], F32, kind="ExternalOutput")
    w_d = nc.dram_tensor("w", [BS, T], F32, kind="ExternalOutput")

    with tile.TileContext(nc) as tc:
        with (
            tc.tile_pool(name="const", bufs=1) as cpool,
            tc.tile_pool(name="xsp", bufs=2) as xs_pool,
            tc.tile_pool(name="pmp", bufs=4) as pm_pool,
            tc.tile_pool(name="thp", bufs=2) as th_pool,
            tc.tile_pool(name="memp", bufs=64) as mem_pool,
            tc.tile_pool(name="stat", bufs=1) as stat_pool,
            tc.tile_pool(name="mm", bufs=3, space="PSUM") as mm_pool,
            tc.tile_pool(name="ep", bufs=2, space="PSUM") as e_pool,
            tc.tile_pool(name="cp", bufs=2, space="PSUM") as ctx_pool,
            tc.tile_pool(name="sp", bufs=1, space="PSUM") as small_pool,
        ):
            wq_sb = cpool.tile([128, KCH, A], F32)
            nc.sync.dma_start(wq_sb[:], wq_d[:].rearrange("(k p) a -> p k a", p=128))
            qt_sb = cpool.tile([128, KCH, BS], F32)
            nc.sync.dma_start(qt_sb[:], qt_d[:].rearrange("(k p) b -> p k b", p=128))
            k2_sb = cpool.tile([KC, A], F32)
            nc.sync.dma_start(k2_sb[:], k2_d[:])
            v_sb = cpool.tile([A, 1], F32)
            nc.sync.dma_start(v_sb[:], v_d[:])
            id_sb = cpool.tile([128, 128], F32)
            nc.sync.dma_start(id_sb[:], id_d[:])

            # pq^T (a, b) = sum_f Wq[f, a] * q[b, f]
            pq_ps = small_pool.tile([A, BS], F32, tag="tp")
            for k in range(KCH):
                nc.tensor.matmul(
                    pq_ps[:], wq_sb[:, k, :], qt_sb[:, k, :],
                    start=(k == 0), stop=(k == KCH - 1),
                )
            pqT = stat_pool.tile([A, BS], F32)
            nc.scalar.copy(pqT[:], pq_ps[:])

            E_all = stat_pool.tile([BS, T], F32)

            for b in range(BS):
                xs_t = xs_pool.tile([KC, T], F32)
                nc.sync.dma_start(xs_t[:], xs_d[b])
                th_t = th_pool.tile([128, T], F32)
                for j in range(NT):
                    pm_t = pm_pool.tile([128, A], F32)
                    nc.sync.dma_start(pm_t[:], pm_d[b, bass.ts(j, 128), :])
                    ps = mm_pool.tile([128, 128], F32)
                    nc.tensor.matmul(
                        ps[:], pm_t[:], id_sb[:], is_transpose=True,
                        start=True, stop=False, skip_group_check=True,
                    )
                    nc.tensor.matmul(
                        ps[:], k2_sb[:], xs_t[:, bass.ts(j, 128)],
                        start=False, stop=True, skip_group_check=True,
                    )
                    nc.scalar.activation(
                        th_t[:, bass.ts(j, 128)], ps[:], AF.Tanh,
                        bias=pqT[:, b:b + 1],
                    )
                for h in range(2):
                    pe = e_pool.tile([1, 512], F32)
                    nc.tensor.matmul(pe[:], v_sb[:], th_t[:, bass.ts(h, 512)])
                    e_st = xs_pool.tile([1, 512], F32, tag="est")
                    nc.scalar.copy(e_st[:], pe[:])
                    nc.sync.dma_start(E_all[b:b + 1, bass.ts(h, 512)], e_st[:])

            # softmax over T per batch (max-subtraction unnecessary: |e|<=sum|v|)
            exp_all = stat_pool.tile([BS, T], F32)
            S = stat_pool.tile([BS, 1], F32)
            nc.scalar.activation(exp_all[:], E_all[:], AF.Exp, accum_out=S[:])
            r = stat_pool.tile([BS, 1], F32)
            nc.vector.reciprocal(r[:], S[:])
            w_sb = stat_pool.tile([BS, T], F32)
            nc.vector.tensor_scalar_mul(w_sb[:], exp_all[:], r[:])
            nc.sync.dma_start(w_d[:], w_sb[:])

            # w columns for the context matvecs: wT[t_in_tile, j, b]
            wT = stat_pool.tile([128, NT, BS], F32)
            for j in range(NT):
                pw = small_pool.tile([128, BS], F32, tag="tp")
                nc.tensor.matmul(
                    pw[:], w_sb[:, bass.ts(j, 128)], id_sb[:BS, :BS],
                    is_transpose=True,
                )
                nc.scalar.copy(wT[:, j, :], pw[:])

            for b in range(BS):
                pc = ctx_pool.tile([1, ENC], F32)
                for j in range(NT):
                    m_t = mem_pool.tile([128, ENC], F32)
                    nc.sync.dma_start(m_t[:], mem_d[b, bass.ts(j, 128), :])
                    nc.tensor.matmul(
                        pc[:], wT[:, j, b:b + 1], m_t[:],
                        start=(j == 0), stop=(j == NT - 1),
                    )
                c_st = xs_pool.tile([1, ENC], F32, tag="cst")
                nc.scalar.copy(c_st[:], pc[:])
                nc.sync.dma_start(ctx_d[b:b + 1, :], c_st[:])
    nc.finalize()
    return nc


def _prep_inputs(query, memory, processed_memory, attention_weights_cat,
                 W_query, conv_kernel, W_loc, v):
    q = np.ascontiguousarray(np.asarray(query, np.float32))
    mem = np.ascontiguousarray(np.asarray(memory, np.float32))
    pm = np.ascontiguousarray(np.asarray(processed_memory, np.float32))
    x = np.ascontiguousarray(np.asarray(attention_weights_cat, np.float32))
    wq = np.ascontiguousarray(np.asarray(W_query, np.float32))
    ck = np.asarray(conv_kernel, np.float32)
    wl = np.asarray(W_loc, np.float32)
    vv = np.ascontiguousarray(np.asarray(v, np.float32).reshape(A, 1))

    k2 = np.einsum("kcf,fa->kca", ck, wl).reshape(KC, A)
    k2 = np.ascontiguousarray(k2.astype(np.float32))

    # im2col of the conv input: xs[b, k*2+c, t] = xpad[b, c, t+k]
    xpad = np.zeros((B, 2, T + 2 * PAD), np.float32)
    xpad[:, :, PAD:PAD + T] = x.transpose(0, 2, 1)
    win = np.lib.stride_tricks.sliding_window_view(xpad, T, axis=2)  # (B,2,KW,T)
    xs = np.ascontiguousarray(win.transpose(0, 2, 1, 3).reshape(B, KC, T))

    ident = np.eye(128, dtype=np.float32)

    in_maps = []
    for c in range(NCORES):
        s = slice(c * BS, (c + 1) * BS)
        in_maps.append({
            "xs": xs[s],
            "pm": pm[s],
            "mem": mem[s],
            "qt": np.ascontiguousarray(q[s].T),
            "wq": wq,
            "k2": k2,
            "v": vv,
            "ident": ident,
        })
    return in_maps


def _run(in_maps, trace=False):
    nc = build_nc()
    return run_bass_kernel_spmd(nc, in_maps, list(range(NCORES)), trace=trace)


def kernel(query, memory, processed_memory, attention_weights_cat,
           W_query, conv_kernel, W_loc, v):
    in_maps = _prep_inputs(query, memory, processed_memory,
                           attention_weights_cat, W_query, conv_kernel, W_loc, v)
    res = _run(in_maps)
    ctx = np.concatenate([res.results[c]["ctx"] for c in range(NCORES)], axis=0)
    w = np.concatenate([res.results[c]["w"] for c in range(NCORES)], axis=0)
    return ctx, w


# revision 7
# speedup vs baseline: 1.6641x; 1.6641x over previous
"""LocationAttention on 8 Trainium2 NeuronCores — pure data parallel over batch.

Math (per batch b):
  pq   = query @ W_query                                  (1, 128)
  loc2 = conv1d(att_cat, conv_kernel, SAME) @ W_loc       (T, 128)
  e    = tanh(pq + loc2 + pm) @ v                         (T,)
  w    = softmax(e)                                       (T,)
  ctx  = w @ memory                                       (512,)

Device layout is A-major (ATT_DIM=128 on partitions):
  - host folds W_loc into the conv kernel (K2 = einsum('kcf,fa->kca')) and
    im2cols the tiny (T,2) conv input into xs[kc,t] so conv+projection is ONE
    matmul per 512 wide T-block: psum[a,t] = K2[kc,a].T @ xs[kc,t]
  - host pre-transposes processed_memory to (B, A, T) so the pm add is a
    plain DVE tensor_add against the conv PSUM (no PE transposes on device)
  - ACT computes tanh(s + pq) with pq as a fused per-partition bias
  - energies: PE matvec v.T @ tanh_T -> (1, T)
  - softmax without max-subtraction (|e| <= sum|v| is small); ACT Exp with
    fused accum_out row-sum; DVE reciprocal + tensor_scalar_mul
  - context: PE matvec w_col.T @ mem_tile accumulated over T tiles
"""

import numpy as np

import concourse.bass as bass
import concourse.tile as tile
from concourse import bacc, mybir
from concourse.bass_utils import run_bass_kernel_spmd

B, T, ENC, QDIM = 128, 1024, 512, 1024
A, F, KW = 128, 32, 31
NCORES = 8
BS = B // NCORES          # 16 batches per core
PAD = KW // 2             # 15
KC = KW * 2               # 62 (tap, channel) pairs
KCH = QDIM // 128         # 8 K-chunks for the query projection
NT = T // 128             # 8 T-tiles per batch
NH = T // 512             # 2 wide T-halves per batch
F32 = mybir.dt.float32
AF = mybir.ActivationFunctionType


def build_nc():
    # Bacc (not plain Bass): its compile() pipeline legalizes multi-sem waits
    # (move_matmul_waits_to_ldweights, event semaphores) which walrus requires.
    nc = bacc.Bacc(None, target_bir_lowering=False)
    xs_d = nc.dram_tensor("xs", [BS, KC, T], F32, kind="ExternalInput")
    pmt_d = nc.dram_tensor("pmt", [BS, A, T], F32, kind="ExternalInput")
    mem_d = nc.dram_tensor("mem", [BS, T, ENC], F32, kind="ExternalInput")
    qt_d = nc.dram_tensor("qt", [QDIM, BS], F32, kind="ExternalInput")
    wq_d = nc.dram_tensor("wq", [QDIM, A], F32, kind="ExternalInput")
    k2_d = nc.dram_tensor("k2", [KC, A], F32, kind="ExternalInput")
    v_d = nc.dram_tensor("v", [A, 1], F32, kind="ExternalInput")
    id_d = nc.dram_tensor("ident", [128, 128], F32, kind="ExternalInput")
    ctx_d = nc.dram_tensor("ctx", [BS, ENC], F32, kind="ExternalOutput")
    w_d = nc.dram_tensor("w", [BS, T], F32, kind="ExternalOutput")

    with tile.TileContext(nc) as tc:
        with (
            tc.tile_pool(name="const", bufs=1) as cpool,
            tc.tile_pool(name="xsp", bufs=2) as xs_pool,
            tc.tile_pool(name="pmp", bufs=4) as pm_pool,
            tc.tile_pool(name="sp1", bufs=3) as s_pool,
            tc.tile_pool(name="thp", bufs=3) as th_pool,
            tc.tile_pool(name="memp", bufs=64) as mem_pool,
            tc.tile_pool(name="stat", bufs=1) as stat_pool,
            tc.tile_pool(name="mm", bufs=3, space="PSUM") as mm_pool,
            tc.tile_pool(name="vp", bufs=3, space="PSUM") as vec_pool,
            tc.tile_pool(name="smp", bufs=1, space="PSUM") as small_pool,
        ):
            wq_sb = cpool.tile([128, KCH, A], F32)
            nc.sync.dma_start(wq_sb[:], wq_d[:].rearrange("(k p) a -> p k a", p=128))
            qt_sb = cpool.tile([128, KCH, BS], F32)
            nc.sync.dma_start(qt_sb[:], qt_d[:].rearrange("(k p) b -> p k b", p=128))
            k2_sb = cpool.tile([KC, A], F32)
            nc.sync.dma_start(k2_sb[:], k2_d[:])
            v_sb = cpool.tile([A, 1], F32)
            nc.sync.dma_start(v_sb[:], v_d[:])
            id_sb = cpool.tile([128, 128], F32)
            nc.sync.dma_start(id_sb[:], id_d[:])

            # pq^T (a, b) = sum_f Wq[f, a] * q[b, f]
            pq_ps = small_pool.tile([A, BS], F32, tag="tp")
            for k in range(KCH):
                nc.tensor.matmul(
                    pq_ps[:], wq_sb[:, k, :], qt_sb[:, k, :],
                    start=(k == 0), stop=(k == KCH - 1),
                )
            pqT = stat_pool.tile([A, BS], F32)
            nc.scalar.copy(pqT[:], pq_ps[:])

            E_all = stat_pool.tile([BS, T], F32)

            for b in range(BS):
                xs_t = xs_pool.tile([KC, T], F32)
                nc.sync.dma_start(xs_t[:], xs_d[b])
                for h in range(NH):
                    pm_t = pm_pool.tile([128, 512], F32)
                    nc.sync.dma_start(pm_t[:], pmt_d[b, :, bass.ts(h, 512)])
                    ps = mm_pool.tile([128, 512], F32)
                    nc.tensor.matmul(ps[:], k2_sb[:], xs_t[:, bass.ts(h, 512)])
                    s_t = s_pool.tile([128, 512], F32)
                    nc.vector.tensor_add(s_t[:], ps[:], pm_t[:])
                    th_t = th_pool.tile([128, 512], F32)
                    nc.scalar.activation(th_t[:], s_t[:], AF.Tanh,
                                         bias=pqT[:, b:b + 1])
                    pe = vec_pool.tile([1, 512], F32, tag="vec")
                    nc.tensor.matmul(pe[:], v_sb[:], th_t[:])
                    e_st = xs_pool.tile([1, 512], F32, tag="est")
                    nc.scalar.copy(e_st[:], pe[:])
                    nc.sync.dma_start(E_all[b:b + 1, bass.ts(h, 512)], e_st[:])

            # softmax over T per batch (max-subtraction unnecessary: |e|<=sum|v|)
            exp_all = stat_pool.tile([BS, T], F32)
            S = stat_pool.tile([BS, 1], F32)
            nc.scalar.activation(exp_all[:], E_all[:], AF.Exp, accum_out=S[:])
            r = stat_pool.tile([BS, 1], F32)
            nc.vector.reciprocal(r[:], S[:])
            w_sb = stat_pool.tile([BS, T], F32)
            nc.vector.tensor_scalar_mul(w_sb[:], exp_all[:], r[:])
            nc.sync.dma_start(w_d[:], w_sb[:])

            # w columns for the context matvecs: wT[t_in_tile, j, b]
            wT = stat_pool.tile([128, NT, BS], F32)
            for j in range(NT):
                pw = small_pool.tile([128, BS], F32, tag="tp")
                nc.tensor.matmul(
                    pw[:], w_sb[:, bass.ts(j, 128)], id_sb[:BS, :BS],
                    is_transpose=True,
                )
                nc.scalar.copy(wT[:, j, :], pw[:])

            for b in range(BS):
                pc = vec_pool.tile([1, ENC], F32, tag="vec")
                for j in range(NT):
                    m_t = mem_pool.tile([128, ENC], F32)
                    nc.sync.dma_start(m_t[:], mem_d[b, bass.ts(j, 128), :])
                    nc.tensor.matmul(
                        pc[:], wT[:, j, b:b + 1], m_t[:],
                        start=(j == 0), stop=(j == NT - 1),
                    )
                c_st = xs_pool.tile([1, ENC], F32, tag="cst")
                nc.scalar.copy(c_st[:], pc[:])
                nc.sync.dma_start(ctx_d[b:b + 1, :], c_st[:])
    nc.finalize()
    return nc


def _prep_inputs(query, memory, processed_memory, attention_weights_cat,
                 W_query, conv_kernel, W_loc, v):
    q = np.ascontiguousarray(np.asarray(query, np.float32))
    mem = np.ascontiguousarray(np.asarray(memory, np.float32))
    pm = np.asarray(processed_memory, np.float32)
    x = np.ascontiguousarray(np.asarray(attention_weights_cat, np.float32))
    wq = np.ascontiguousarray(np.asarray(W_query, np.float32))
    ck = np.asarray(conv_kernel, np.float32)
    wl = np.asarray(W_loc, np.float32)
    vv = np.ascontiguousarray(np.asarray(v, np.float32).reshape(A, 1))

    k2 = np.einsum("kcf,fa->kca", ck, wl).reshape(KC, A)
    k2 = np.ascontiguousarray(k2.astype(np.float32))

    # A-major processed_memory so the device never transposes it
    pmt = np.ascontiguousarray(pm.transpose(0, 2, 1))  # (B, A, T)

    # im2col of the conv input: xs[b, k*2+c, t] = xpad[b, c, t+k]
    xpad = np.zeros((B, 2, T + 2 * PAD), np.float32)
    xpad[:, :, PAD:PAD + T] = x.transpose(0, 2, 1)
    win = np.lib.stride_tricks.sliding_window_view(xpad, T, axis=2)  # (B,2,KW,T)
    xs = np.ascontiguousarray(win.transpose(0, 2, 1, 3).reshape(B, KC, T))

    ident = np.eye(128, dtype=np.float32)

    in_maps = []
    for c in range(NCORES):
        s = slice(c * BS, (c + 1) * BS)
        in_maps.append({
            "xs": xs[s],
            "pmt": pmt[s],
            "mem": mem[s],
            "qt": np.ascontiguousarray(q[s].T),
            "wq": wq,
            "k2": k2,
            "v": vv,
            "ident": ident,
        })
    return in_maps


def _run(in_maps, trace=False):
    nc = build_nc()
    return run_bass_kernel_spmd(nc, in_maps, list(range(NCORES)), trace=trace)


def kernel(query, memory, processed_memory, attention_weights_cat,
           W_query, conv_kernel, W_loc, v):
    in_maps = _prep_inputs(query, memory, processed_memory,
                           attention_weights_cat, W_query, conv_kernel, W_loc, v)
    res = _run(in_maps)
    ctx = np.concatenate([res.results[c]["ctx"] for c in range(NCORES)], axis=0)
    w = np.concatenate([res.results[c]["w"] for c in range(NCORES)], axis=0)
    return ctx, w
